# revision 3
# baseline (speedup 1.0000x reference)
"""DFINE post-processor Trainium2 kernel (nn_DFINEPostProcessor).

Inputs (full): pred_logits [256,1000,80] f32, pred_boxes [256,1000,4] f32,
orig_target_sizes [256,2] f32, num_top_queries=300.
Returns (labels [256,300] i32, boxes [256,300,4] f32, scores [256,300] f32)
matching jax.lax.top_k ordering exactly (desc value, asc flat index ties).

Sharding: pure data parallel, 8 NeuronCores x 32 images each. Per core:

  P1  DVE max8+find_index8 per partition-half (313/312) of each image's
      [128, 625] logits: per-half sorted top-8 values + indices.
  P2  Rows = (img, 32-partition group) [128, 512]; 14 rounds of
      (max8, find_index8, match_replace) -> sorted top-112 per row.
  P3  Rows = img [32, 448]; 38 rounds -> sorted top-304 + positions.
      The DVE triplet implements exact stable descending sort
      (duplicates get distinct ascending indices, verified on HW), so
      tie order matches jax end to end.
  EP  Position-chain dereference via gpsimd ap_gather. Index arrays are
      pi-permuted with one strided DVE copy so the 16-partition "wrap"
      DMA is contiguous and gather outputs return in rank order.
      Integer math uses the 1.5*2^23 magic-number floor (exact over the
      needed domains). Boxes are decoded+scaled with ops that round
      identically to the reference, then gathered per image.
"""

import numpy as np

B, Q, C = 256, 1000, 80
NCORES = 8
NIMG = B // NCORES  # 32
K_OUT = 300
K_PAD = 304  # 19*16: ap_gather num_idxs must be a multiple of 16
HALF = 313  # 625 = 313 + 312
R2 = 14  # P2 rounds -> sorted 112 per 32-partition group
R3 = 38  # P3 rounds -> sorted 304 per image
W2 = 8 * R2  # 112
W3 = 4 * W2  # 448
NEG = -3.0e38
MAGIC = 12582912.0  # 1.5 * 2^23
DHALF = -0.5 + 2.0**-9

_CACHE: dict = {}


def _build():
    import concourse.bacc as bacc
    import concourse.bass_isa as bass_isa
    import concourse.mybir as mybir
    import concourse.tile as tile

    f32 = mybir.dt.float32
    u16 = mybir.dt.uint16
    i16 = mybir.dt.int16
    i32 = mybir.dt.int32
    Alu = mybir.AluOpType
    Act = mybir.ActivationFunctionType

    nc = bacc.Bacc("TRN2", target_bir_lowering=False, debug=False)

    d_log = nc.dram_tensor("logits", [NIMG, 128, 625], f32, kind="ExternalInput")
    d_box = nc.dram_tensor("boxes", [NIMG, Q, 4], f32, kind="ExternalInput")
    d_siz = nc.dram_tensor("sizes", [NIMG, 2], f32, kind="ExternalInput")
    d_lab = nc.dram_tensor("out_labels", [NIMG, K_PAD], i32, kind="ExternalOutput")
    d_sco = nc.dram_tensor("out_scores", [NIMG, K_PAD], f32, kind="ExternalOutput")
    d_obx = nc.dram_tensor("out_boxes", [NIMG, K_OUT, 4], f32, kind="ExternalOutput")

    def ap_gather(out_ap, in_ap, idxs_ap, num_elems):
        g = nc.gpsimd
        return g.add_instruction(
            bass_isa.InstAPGather(
                name=f"I-{nc.next_id()}",
                ins=[
                    g.lower_ap(in_ap, for_isa=True),
                    g.lower_ap(idxs_ap, for_isa=True),
                ],
                outs=[g.lower_ap(out_ap, for_isa=True)],
                _channels=128,
                _num_elems=num_elems,
                _d=1,
                _num_idxs=K_PAD,
            )
        )

    with tile.TileContext(nc) as tc:
        with (
            tc.tile_pool(name="xin", bufs=2) as xin,
            tc.tile_pool(name="wk", bufs=1) as wk,
        ):
            # ---------------- P1: load + per-half extraction ----------------
            v16 = wk.tile([128, 16 * NIMG], f32)
            i16t = wk.tile([128, 16 * NIMG], u16)
            for m in range(NIMG):
                x = xin.tile([128, 625], f32, name=f"x{m}", tag="x")
                nc.sync.dma_start(out=x[:], in_=d_log[m])
                cv = v16[:, 16 * m : 16 * m + 16]
                ci = i16t[:, 16 * m : 16 * m + 16]
                nc.vector.max(cv[:, 0:8], x[:, 0:HALF])
                nc.vector.max(cv[:, 8:16], x[:, HALF:625])
                nc.vector.max_index(ci[:, 0:8], cv[:, 0:8], x[:, 0:HALF])
                nc.vector.max_index(ci[:, 8:16], cv[:, 8:16], x[:, HALF:625])
            i16f = wk.tile([128, 16 * NIMG], f32)
            nc.vector.tensor_copy(i16f[:], i16t[:])

            # ---------------- P2 ----------------
            v2a = wk.tile([128, 512], f32)
            v2b = wk.tile([128, 512], f32)
            s2v = wk.tile([128, W2], f32)
            s2p = wk.tile([128, W2], u16)
            for m in range(NIMG):
                # [128,16] (p = g*32+pg, c) -> partitions 4m+g, free pg*16+c
                nc.scalar.dma_start(
                    out=v2a[4 * m : 4 * m + 4, :], in_=v16[:, 16 * m : 16 * m + 16]
                )
            for r in range(R2):
                cur = v2a if r % 2 == 0 else v2b
                nxt = v2b if r % 2 == 0 else v2a
                nc.vector.max(s2v[:, 8 * r : 8 * r + 8], cur[:])
                nc.vector.max_index(
                    s2p[:, 8 * r : 8 * r + 8], s2v[:, 8 * r : 8 * r + 8], cur[:]
                )
                nc.vector.match_replace(nxt[:], s2v[:, 8 * r : 8 * r + 8], cur[:], NEG)
            s2pf = wk.tile([128, W2], f32)
            nc.vector.tensor_copy(s2pf[:], s2p[:])

            # ---------------- P3 ----------------
            v3a = wk.tile([32, W3], f32)
            v3b = wk.tile([32, W3], f32)
            p448 = wk.tile([32, W3], f32)
            s3v = wk.tile([32, 8 * R3], f32)
            s3p = wk.tile([32, 8 * R3], u16)
            nc.sync.dma_start(out=v3a[:], in_=s2v[:])
            nc.sync.dma_start(out=p448[:], in_=s2pf[:])
            for r in range(R3):
                cur = v3a if r % 2 == 0 else v3b
                nxt = v3b if r % 2 == 0 else v3a
                nc.vector.max(s3v[:, 8 * r : 8 * r + 8], cur[:])
                nc.vector.max_index(
                    s3p[:, 8 * r : 8 * r + 8], s3v[:, 8 * r : 8 * r + 8], cur[:]
                )
                nc.vector.match_replace(nxt[:], s3v[:, 8 * r : 8 * r + 8], cur[:], NEG)

            def pi_i16(name, src_f32):
                """rank-order f32 [32, K_PAD] -> pi-permuted int16 for wrap DMAs."""
                pf = wk.tile([32, K_PAD], f32, name=f"{name}f")
                nc.vector.tensor_copy(pf[:], src_f32.rearrange("a (s x) -> a x s", x=16))
                pi = wk.tile([32, K_PAD], i16, name=f"{name}i")
                nc.vector.tensor_copy(pi[:], pf[:])
                return pi

            def floor_div(name, src, inv):
                t = wk.tile([32, K_PAD], f32, name=name)
                nc.vector.tensor_scalar(t[:], src[:], inv, DHALF, Alu.mult, Alu.add)
                nc.vector.tensor_scalar(t[:], t[:], MAGIC, None, Alu.add)
                nc.vector.tensor_scalar(t[:], t[:], MAGIC, None, Alu.subtract)
                return t

            # ---------------- EP gather 1: P3 pos -> P2 pos ----------------
            p3f = wk.tile([32, K_PAD], f32)
            nc.vector.tensor_copy(p3f[:], s3p[:])
            w1i = pi_i16("w1", p3f[:])
            e1 = wk.tile([32, K_PAD], f32)
            for cl in range(4):
                r0 = 8 * cl
                w1 = wk.tile([128, 19], i16, name=f"w1_{cl}")
                nc.sync.dma_start(out=w1[:], in_=w1i[r0 : r0 + 8, :])
                t448 = wk.tile([128, W3], f32, name=f"t448_{cl}")
                nc.gpsimd.memset(t448[:], 0.0)
                nc.scalar.dma_start(out=t448[0:128:16, :], in_=p448[r0 : r0 + 8, :])
                o1 = wk.tile([128, K_PAD], f32, name=f"o1_{cl}")
                ap_gather(o1[:], t448[:], w1[:], num_elems=W3)
                nc.sync.dma_start(out=e1[r0 : r0 + 8, :], in_=o1[0:128:16, :])

            # math-1: g = p3 // 112 ; s = g*512 + e1
            gg = floor_div("gg", p3f, 1.0 / W2)
            ss = wk.tile([32, K_PAD], f32)
            nc.vector.scalar_tensor_tensor(ss[:], gg[:], 512.0, e1[:], Alu.mult, Alu.add)
            w2i = pi_i16("w2", ss[:])

            # ---------------- EP gather 2: slot -> in-half index ----------------
            e2 = wk.tile([32, K_PAD], f32)
            for cl in range(4):
                r0 = 8 * cl
                w2 = wk.tile([128, 19], i16, name=f"w2_{cl}")
                nc.sync.dma_start(out=w2[:], in_=w2i[r0 : r0 + 8, :])
                t1 = wk.tile([128, 2048], f32, name=f"t1_{cl}")
                nc.gpsimd.memset(t1[:], 0.0)
                for gi in range(8):
                    m = r0 + gi
                    nc.scalar.dma_start(
                        out=t1[16 * gi : 16 * gi + 1, :],
                        in_=i16f[:, 16 * m : 16 * m + 16],
                    )
                o2 = wk.tile([128, K_PAD], f32, name=f"o2_{cl}")
                ap_gather(o2[:], t1[:], w2[:], num_elems=2048)
                nc.sync.dma_start(out=e2[r0 : r0 + 8, :], in_=o2[0:128:16, :])

            # math-2: flat = (s//16)*625 + (s%16>=8)*313 + j ; labels ; q
            pq = floor_div("pq", ss, 1.0 / 16.0)
            ccm = wk.tile([32, K_PAD], f32)
            nc.vector.scalar_tensor_tensor(
                ccm[:], pq[:], -16.0, ss[:], Alu.mult, Alu.add
            )
            hh = wk.tile([32, K_PAD], f32)
            nc.vector.tensor_scalar(hh[:], ccm[:], 8.0, None, Alu.is_ge)
            flat = wk.tile([32, K_PAD], f32)
            nc.vector.scalar_tensor_tensor(
                flat[:], pq[:], 625.0, e2[:], Alu.mult, Alu.add
            )
            nc.vector.scalar_tensor_tensor(
                flat[:], hh[:], float(HALF), flat[:], Alu.mult, Alu.add
            )
            qf = floor_div("qf", flat, 1.0 / C)
            lab = wk.tile([32, K_PAD], f32)
            nc.vector.scalar_tensor_tensor(
                lab[:], qf[:], -float(C), flat[:], Alu.mult, Alu.add
            )
            labi = wk.tile([32, K_PAD], i32)
            nc.vector.tensor_copy(labi[:], lab[:])
            nc.sync.dma_start(out=d_lab[:], in_=labi[:])
            w3i = pi_i16("w3", qf[:])

            # scores
            sc = wk.tile([32, K_PAD], f32)
            nc.scalar.activation(sc[:], s3v[:], Act.Sigmoid)
            nc.sync.dma_start(out=d_sco[:], in_=sc[:])

            # ---------------- EP: boxes ----------------
            cx = wk.tile([32, Q], f32)
            cy = wk.tile([32, Q], f32)
            wd = wk.tile([32, Q], f32)
            ht = wk.tile([32, Q], f32)
            nc.sync.dma_start(out=cx[:], in_=d_box[:, :, 0])
            nc.sync.dma_start(out=cy[:], in_=d_box[:, :, 1])
            nc.sync.dma_start(out=wd[:], in_=d_box[:, :, 2])
            nc.sync.dma_start(out=ht[:], in_=d_box[:, :, 3])
            sz = wk.tile([32, 2], f32)
            nc.sync.dma_start(out=sz[:], in_=d_siz[:])
            pl = []
            for j, (cn, wh, sgn, sci) in enumerate(
                [(cx, wd, -0.5, 0), (cy, ht, -0.5, 1), (cx, wd, 0.5, 0), (cy, ht, 0.5, 1)]
            ):
                u = wk.tile([32, Q], f32, name=f"u{j}")
                nc.vector.scalar_tensor_tensor(
                    u[:], wh[:], sgn, cn[:], Alu.mult, Alu.add
                )
                v = wk.tile([32, Q], f32, name=f"pl{j}")
                nc.scalar.activation(v[:], u[:], Act.Copy, scale=sz[:, sci : sci + 1])
                pl.append(v)
            for cl in range(4):
                r0 = 8 * cl
                w3 = wk.tile([128, 19], i16, name=f"w3_{cl}")
                nc.sync.dma_start(out=w3[:], in_=w3i[r0 : r0 + 8, :])
                pt = wk.tile([128, Q], f32, name=f"pt_{cl}")
                nc.gpsimd.memset(pt[:], 0.0)
                for j in range(4):
                    nc.scalar.dma_start(out=pt[j : 128 : 16, :], in_=pl[j][r0 : r0 + 8, :])
                o3 = wk.tile([128, K_PAD], f32, name=f"o3_{cl}")
                ap_gather(o3[:], pt[:], w3[:], num_elems=Q)
                for j in range(4):
                    nc.sync.dma_start(
                        out=d_obx[r0 : r0 + 8, 0:K_OUT, j],
                        in_=o3[j : 128 : 16, 0:K_OUT],
                    )

    nc.compile()
    return nc


def _get_nc():
    if "nc" not in _CACHE:
        _CACHE["nc"] = _build()
    return _CACHE["nc"]


def kernel(pred_logits, pred_boxes, orig_target_sizes, num_top_queries):
    from concourse.bass_utils import run_bass_kernel_spmd

    assert int(num_top_queries) == K_OUT
    pred_logits = np.ascontiguousarray(np.asarray(pred_logits, dtype=np.float32))
    pred_boxes = np.ascontiguousarray(np.asarray(pred_boxes, dtype=np.float32))
    sizes = np.ascontiguousarray(np.asarray(orig_target_sizes, dtype=np.float32))

    nc = _get_nc()
    in_maps = []
    for c in range(NCORES):
        sl = slice(c * NIMG, (c + 1) * NIMG)
        in_maps.append(
            {
                "logits": pred_logits[sl].reshape(NIMG, 128, 625),
                "boxes": pred_boxes[sl],
                "sizes": sizes[sl],
            }
        )
    res = run_bass_kernel_spmd(nc, in_maps, list(range(NCORES))).results

    labels = np.concatenate([r["out_labels"][:, :K_OUT] for r in res], axis=0)
    scores = np.concatenate([r["out_scores"][:, :K_OUT] for r in res], axis=0)
    boxes = np.concatenate([r["out_boxes"] for r in res], axis=0)
    return labels.astype(np.int32), boxes, scores


# revision 5
# speedup vs baseline: 1.2100x; 1.2100x over previous
"""DFINE post-processor Trainium2 kernel (nn_DFINEPostProcessor).

Inputs (full): pred_logits [256,1000,80] f32, pred_boxes [256,1000,4] f32,
orig_target_sizes [256,2] f32, num_top_queries=300.
Returns (labels [256,300] i32, boxes [256,300,4] f32, scores [256,300] f32)
matching jax.lax.top_k ordering exactly (desc value, asc flat index ties).

Sharding: pure data parallel, 8 NeuronCores x 32 images each. Per core:

  P1  DVE max8+find_index8 per partition-half (313/312) of each image's
      [128, 625] logits: per-half sorted top-8 values + indices.
  P2  Rows = (img, 32-partition group) [128, 512]; 14 rounds of
      (max8, find_index8, match_replace) -> sorted top-112 per row.
  P3  Rows = img [32, 448]; 38 rounds -> sorted top-304 + positions.
      The DVE triplet implements exact stable descending sort
      (duplicates get distinct ascending indices, verified on HW), so
      tie order matches jax end to end.
  EP  Position-chain dereference via gpsimd ap_gather. Index arrays are
      pi-permuted with one strided DVE copy so the 16-partition "wrap"
      DMA is contiguous and gather outputs return in rank order.
      Integer math uses the 1.5*2^23 magic-number floor (exact over the
      needed domains). Boxes are decoded+scaled with ops that round
      identically to the reference, then gathered per image.
"""

import numpy as np

B, Q, C = 256, 1000, 80
NCORES = 8
NIMG = B // NCORES  # 32
K_OUT = 300
K_PAD = 304  # 19*16: ap_gather num_idxs must be a multiple of 16
HALF = 313  # 625 = 313 + 312
R2 = 14  # P2 rounds -> sorted 112 per 32-partition group
R3 = 38  # P3 rounds -> sorted 304 per image
W2 = 8 * R2  # 112
W3 = 4 * W2  # 448
NEG = -3.0e38
MAGIC = 12582912.0  # 1.5 * 2^23
DHALF = -0.5 + 2.0**-9

_CACHE: dict = {}


def _build():
    import concourse.bacc as bacc
    import concourse.bass_isa as bass_isa
    import concourse.mybir as mybir
    import concourse.tile as tile

    f32 = mybir.dt.float32
    u16 = mybir.dt.uint16
    i16 = mybir.dt.int16
    i32 = mybir.dt.int32
    Alu = mybir.AluOpType
    Act = mybir.ActivationFunctionType

    nc = bacc.Bacc("TRN2", target_bir_lowering=False, debug=False)

    d_log = nc.dram_tensor("logits", [NIMG, 128, 625], f32, kind="ExternalInput")
    d_box = nc.dram_tensor("boxes", [NIMG, Q, 4], f32, kind="ExternalInput")
    d_siz = nc.dram_tensor("sizes", [NIMG, 2], f32, kind="ExternalInput")
    d_lab = nc.dram_tensor("out_labels", [NIMG, K_PAD], i32, kind="ExternalOutput")
    d_sco = nc.dram_tensor("out_scores", [NIMG, K_PAD], f32, kind="ExternalOutput")
    d_obx = nc.dram_tensor("out_boxes", [NIMG, K_OUT, 4], f32, kind="ExternalOutput")

    def ap_gather(out_ap, in_ap, idxs_ap, num_elems):
        g = nc.gpsimd
        return g.add_instruction(
            bass_isa.InstAPGather(
                name=f"I-{nc.next_id()}",
                ins=[
                    g.lower_ap(in_ap, for_isa=True),
                    g.lower_ap(idxs_ap, for_isa=True),
                ],
                outs=[g.lower_ap(out_ap, for_isa=True)],
                _channels=128,
                _num_elems=num_elems,
                _d=1,
                _num_idxs=K_PAD,
            )
        )

    with tile.TileContext(nc) as tc:
        with (
            tc.tile_pool(name="xin", bufs=6) as xin,
            tc.tile_pool(name="wk", bufs=1) as wk,
        ):
            # ---------------- P1: load + per-half extraction ----------------
            v16 = wk.tile([128, 16 * NIMG], f32)
            i16t = wk.tile([128, 16 * NIMG], u16)
            load_engines = [nc.sync, nc.scalar, nc.gpsimd]
            for m in range(NIMG):
                x = xin.tile([128, 625], f32, name=f"x{m}", tag="x")
                load_engines[m % 3].dma_start(out=x[:], in_=d_log[m])
                cv = v16[:, 16 * m : 16 * m + 16]
                ci = i16t[:, 16 * m : 16 * m + 16]
                nc.vector.max(cv[:, 0:8], x[:, 0:HALF])
                nc.vector.max(cv[:, 8:16], x[:, HALF:625])
                nc.vector.max_index(ci[:, 0:8], cv[:, 0:8], x[:, 0:HALF])
                nc.vector.max_index(ci[:, 8:16], cv[:, 8:16], x[:, HALF:625])
            i16f = wk.tile([128, 16 * NIMG], f32)
            nc.vector.tensor_copy(i16f[:], i16t[:])

            # ---------------- P2 ----------------
            v2a = wk.tile([128, 512], f32)
            v2b = wk.tile([128, 512], f32)
            s2v = wk.tile([128, W2], f32)
            s2p = wk.tile([128, W2], u16)
            re_engines = [nc.gpsimd, nc.sync]
            for m in range(NIMG):
                # [128,16] (p = g*32+pg, c) -> partitions 4m+g, free pg*16+c
                re_engines[m % 2].dma_start(
                    out=v2a[4 * m : 4 * m + 4, :], in_=v16[:, 16 * m : 16 * m + 16]
                )
            for r in range(R2):
                cur = v2a if r % 2 == 0 else v2b
                nxt = v2b if r % 2 == 0 else v2a
                nc.vector.max(s2v[:, 8 * r : 8 * r + 8], cur[:])
                nc.vector.max_index(
                    s2p[:, 8 * r : 8 * r + 8], s2v[:, 8 * r : 8 * r + 8], cur[:]
                )
                nc.vector.match_replace(nxt[:], s2v[:, 8 * r : 8 * r + 8], cur[:], NEG)
            s2pf = wk.tile([128, W2], f32)
            nc.vector.tensor_copy(s2pf[:], s2p[:])

            # ---------------- P3 ----------------
            v3a = wk.tile([32, W3], f32)
            v3b = wk.tile([32, W3], f32)
            p448 = wk.tile([32, W3], f32)
            s3v = wk.tile([32, 8 * R3], f32)
            s3p = wk.tile([32, 8 * R3], u16)
            nc.sync.dma_start(out=v3a[:], in_=s2v[:])
            nc.sync.dma_start(out=p448[:], in_=s2pf[:])
            for r in range(R3):
                cur = v3a if r % 2 == 0 else v3b
                nxt = v3b if r % 2 == 0 else v3a
                nc.vector.max(s3v[:, 8 * r : 8 * r + 8], cur[:])
                nc.vector.max_index(
                    s3p[:, 8 * r : 8 * r + 8], s3v[:, 8 * r : 8 * r + 8], cur[:]
                )
                nc.vector.match_replace(nxt[:], s3v[:, 8 * r : 8 * r + 8], cur[:], NEG)

            def pi_i16(name, src_f32):
                """rank-order f32 [32, K_PAD] -> pi-permuted int16 for wrap DMAs."""
                pf = wk.tile([32, K_PAD], f32, name=f"{name}f")
                nc.vector.tensor_copy(pf[:], src_f32.rearrange("a (s x) -> a x s", x=16))
                pi = wk.tile([32, K_PAD], i16, name=f"{name}i")
                nc.vector.tensor_copy(pi[:], pf[:])
                return pi

            def floor_div(name, src, inv):
                t = wk.tile([32, K_PAD], f32, name=name)
                nc.vector.tensor_scalar(t[:], src[:], inv, DHALF, Alu.mult, Alu.add)
                nc.vector.tensor_scalar(t[:], t[:], MAGIC, None, Alu.add)
                nc.vector.tensor_scalar(t[:], t[:], MAGIC, None, Alu.subtract)
                return t

            # ---------------- EP gather 1: P3 pos -> P2 pos ----------------
            p3f = wk.tile([32, K_PAD], f32)
            nc.vector.tensor_copy(p3f[:], s3p[:])
            w1i = pi_i16("w1", p3f[:])
            e1 = wk.tile([32, K_PAD], f32)
            for cl in range(4):
                r0 = 8 * cl
                w1 = wk.tile([128, 19], i16, name=f"w1_{cl}")
                nc.sync.dma_start(out=w1[:], in_=w1i[r0 : r0 + 8, :])
                t448 = wk.tile([128, W3], f32, name=f"t448_{cl}")
                nc.gpsimd.memset(t448[:], 0.0)
                nc.scalar.dma_start(out=t448[0:128:16, :], in_=p448[r0 : r0 + 8, :])
                o1 = wk.tile([128, K_PAD], f32, name=f"o1_{cl}")
                ap_gather(o1[:], t448[:], w1[:], num_elems=W3)
                nc.sync.dma_start(out=e1[r0 : r0 + 8, :], in_=o1[0:128:16, :])

            # math-1: g = p3 // 112 ; s = g*512 + e1
            gg = floor_div("gg", p3f, 1.0 / W2)
            ss = wk.tile([32, K_PAD], f32)
            nc.vector.scalar_tensor_tensor(ss[:], gg[:], 512.0, e1[:], Alu.mult, Alu.add)
            w2i = pi_i16("w2", ss[:])

            # ---------------- EP gather 2: slot -> in-half index ----------------
            e2 = wk.tile([32, K_PAD], f32)
            for cl in range(4):
                r0 = 8 * cl
                w2 = wk.tile([128, 19], i16, name=f"w2_{cl}")
                nc.sync.dma_start(out=w2[:], in_=w2i[r0 : r0 + 8, :])
                t1 = wk.tile([128, 2048], f32, name=f"t1_{cl}")
                nc.gpsimd.memset(t1[:], 0.0)
                for gi in range(8):
                    m = r0 + gi
                    (nc.scalar if gi % 2 == 0 else nc.gpsimd).dma_start(
                        out=t1[16 * gi : 16 * gi + 1, :],
                        in_=i16f[:, 16 * m : 16 * m + 16],
                    )
                o2 = wk.tile([128, K_PAD], f32, name=f"o2_{cl}")
                ap_gather(o2[:], t1[:], w2[:], num_elems=2048)
                nc.sync.dma_start(out=e2[r0 : r0 + 8, :], in_=o2[0:128:16, :])

            # math-2: flat = (s//16)*625 + (s%16>=8)*313 + j ; labels ; q
            pq = floor_div("pq", ss, 1.0 / 16.0)
            ccm = wk.tile([32, K_PAD], f32)
            nc.vector.scalar_tensor_tensor(
                ccm[:], pq[:], -16.0, ss[:], Alu.mult, Alu.add
            )
            hh = wk.tile([32, K_PAD], f32)
            nc.vector.tensor_scalar(hh[:], ccm[:], 8.0, None, Alu.is_ge)
            flat = wk.tile([32, K_PAD], f32)
            nc.vector.scalar_tensor_tensor(
                flat[:], pq[:], 625.0, e2[:], Alu.mult, Alu.add
            )
            nc.vector.scalar_tensor_tensor(
                flat[:], hh[:], float(HALF), flat[:], Alu.mult, Alu.add
            )
            qf = floor_div("qf", flat, 1.0 / C)
            lab = wk.tile([32, K_PAD], f32)
            nc.vector.scalar_tensor_tensor(
                lab[:], qf[:], -float(C), flat[:], Alu.mult, Alu.add
            )
            labi = wk.tile([32, K_PAD], i32)
            nc.vector.tensor_copy(labi[:], lab[:])
            nc.sync.dma_start(out=d_lab[:], in_=labi[:])
            w3i = pi_i16("w3", qf[:])

            # scores
            sc = wk.tile([32, K_PAD], f32)
            nc.scalar.activation(sc[:], s3v[:], Act.Sigmoid)
            nc.sync.dma_start(out=d_sco[:], in_=sc[:])

            # ---------------- EP: boxes ----------------
            cx = wk.tile([32, Q], f32)
            cy = wk.tile([32, Q], f32)
            wd = wk.tile([32, Q], f32)
            ht = wk.tile([32, Q], f32)
            nc.sync.dma_start(out=cx[:], in_=d_box[:, :, 0])
            nc.sync.dma_start(out=cy[:], in_=d_box[:, :, 1])
            nc.sync.dma_start(out=wd[:], in_=d_box[:, :, 2])
            nc.sync.dma_start(out=ht[:], in_=d_box[:, :, 3])
            sz = wk.tile([32, 2], f32)
            nc.sync.dma_start(out=sz[:], in_=d_siz[:])
            pl = []
            for j, (cn, wh, sgn, sci) in enumerate(
                [(cx, wd, -0.5, 0), (cy, ht, -0.5, 1), (cx, wd, 0.5, 0), (cy, ht, 0.5, 1)]
            ):
                u = wk.tile([32, Q], f32, name=f"u{j}")
                nc.vector.scalar_tensor_tensor(
                    u[:], wh[:], sgn, cn[:], Alu.mult, Alu.add
                )
                v = wk.tile([32, Q], f32, name=f"pl{j}")
                nc.scalar.activation(v[:], u[:], Act.Copy, scale=sz[:, sci : sci + 1])
                pl.append(v)
            for cl in range(4):
                r0 = 8 * cl
                w3 = wk.tile([128, 19], i16, name=f"w3_{cl}")
                nc.sync.dma_start(out=w3[:], in_=w3i[r0 : r0 + 8, :])
                pt = wk.tile([128, Q], f32, name=f"pt_{cl}")
                nc.gpsimd.memset(pt[:], 0.0)
                for j in range(4):
                    nc.scalar.dma_start(out=pt[j : 128 : 16, :], in_=pl[j][r0 : r0 + 8, :])
                o3 = wk.tile([128, K_PAD], f32, name=f"o3_{cl}")
                ap_gather(o3[:], pt[:], w3[:], num_elems=Q)
                for j in range(4):
                    nc.sync.dma_start(
                        out=d_obx[r0 : r0 + 8, 0:K_OUT, j],
                        in_=o3[j : 128 : 16, 0:K_OUT],
                    )

    nc.compile()
    return nc


def _get_nc():
    if "nc" not in _CACHE:
        _CACHE["nc"] = _build()
    return _CACHE["nc"]


def kernel(pred_logits, pred_boxes, orig_target_sizes, num_top_queries):
    from concourse.bass_utils import run_bass_kernel_spmd

    assert int(num_top_queries) == K_OUT
    pred_logits = np.ascontiguousarray(np.asarray(pred_logits, dtype=np.float32))
    pred_boxes = np.ascontiguousarray(np.asarray(pred_boxes, dtype=np.float32))
    sizes = np.ascontiguousarray(np.asarray(orig_target_sizes, dtype=np.float32))

    nc = _get_nc()
    in_maps = []
    for c in range(NCORES):
        sl = slice(c * NIMG, (c + 1) * NIMG)
        in_maps.append(
            {
                "logits": pred_logits[sl].reshape(NIMG, 128, 625),
                "boxes": pred_boxes[sl],
                "sizes": sizes[sl],
            }
        )
    res = run_bass_kernel_spmd(nc, in_maps, list(range(NCORES))).results

    labels = np.concatenate([r["out_labels"][:, :K_OUT] for r in res], axis=0)
    scores = np.concatenate([r["out_scores"][:, :K_OUT] for r in res], axis=0)
    boxes = np.concatenate([r["out_boxes"] for r in res], axis=0)
    return labels.astype(np.int32), boxes, scores


# revision 7
# speedup vs baseline: 1.2119x; 1.0016x over previous
"""DFINE post-processor Trainium2 kernel (nn_DFINEPostProcessor).

Inputs (full): pred_logits [256,1000,80] f32, pred_boxes [256,1000,4] f32,
orig_target_sizes [256,2] f32, num_top_queries=300.
Returns (labels [256,300] i32, boxes [256,300,4] f32, scores [256,300] f32)
matching jax.lax.top_k ordering exactly (desc value, asc flat index ties).

Sharding: pure data parallel, 8 NeuronCores x 32 images each. Per core:

  P1  DVE max8+find_index8 per partition-half (313/312) of each image's
      [128, 625] logits: per-half sorted top-8 values + indices.
  P2  Rows = (img, 32-partition group) [128, 512]; 14 rounds of
      (max8, find_index8, match_replace) -> sorted top-112 per row.
  P3  Rows = img [32, 448]; 38 rounds -> sorted top-304 + positions.
      The DVE triplet implements exact stable descending sort
      (duplicates get distinct ascending indices, verified on HW), so
      tie order matches jax end to end.
  EP  Position-chain dereference via gpsimd ap_gather. Index arrays are
      pi-permuted with one strided DVE copy so the 16-partition "wrap"
      DMA is contiguous and gather outputs return in rank order.
      Integer math uses the 1.5*2^23 magic-number floor (exact over the
      needed domains). Boxes are decoded+scaled with ops that round
      identically to the reference, then gathered per image.
"""

import numpy as np

B, Q, C = 256, 1000, 80
NCORES = 8
NIMG = B // NCORES  # 32
K_OUT = 300
K_PAD = 304  # 19*16: ap_gather num_idxs must be a multiple of 16
HALF = 313  # 625 = 313 + 312
R2 = 14  # P2 rounds -> sorted 112 per 32-partition group
R3 = 38  # P3 rounds -> sorted 304 per image
W2 = 8 * R2  # 112
W3 = 4 * W2  # 448
NEG = -3.0e38
MAGIC = 12582912.0  # 1.5 * 2^23
DHALF = -0.5 + 2.0**-9

_CACHE: dict = {}


def _build():
    import concourse.bacc as bacc
    import concourse.bass_isa as bass_isa
    import concourse.mybir as mybir
    import concourse.tile as tile

    f32 = mybir.dt.float32
    u16 = mybir.dt.uint16
    i16 = mybir.dt.int16
    i32 = mybir.dt.int32
    Alu = mybir.AluOpType
    Act = mybir.ActivationFunctionType

    nc = bacc.Bacc("TRN2", target_bir_lowering=False, debug=False)

    d_log = nc.dram_tensor("logits", [NIMG, 128, 625], f32, kind="ExternalInput")
    d_box = nc.dram_tensor("boxes", [NIMG, Q, 4], f32, kind="ExternalInput")
    d_siz = nc.dram_tensor("sizes", [NIMG, 2], f32, kind="ExternalInput")
    d_lab = nc.dram_tensor("out_labels", [NIMG, K_PAD], i32, kind="ExternalOutput")
    d_sco = nc.dram_tensor("out_scores", [NIMG, K_PAD], f32, kind="ExternalOutput")
    d_obx = nc.dram_tensor("out_boxes", [NIMG, K_OUT, 4], f32, kind="ExternalOutput")

    def ap_gather(out_ap, in_ap, idxs_ap, num_elems):
        g = nc.gpsimd
        return g.add_instruction(
            bass_isa.InstAPGather(
                name=f"I-{nc.next_id()}",
                ins=[
                    g.lower_ap(in_ap, for_isa=True),
                    g.lower_ap(idxs_ap, for_isa=True),
                ],
                outs=[g.lower_ap(out_ap, for_isa=True)],
                _channels=128,
                _num_elems=num_elems,
                _d=1,
                _num_idxs=K_PAD,
            )
        )

    _dq_state = [0]

    def dq():
        _dq_state[0] ^= 1
        return nc.scalar if _dq_state[0] else nc.gpsimd

    with tile.TileContext(nc) as tc:
        with (
            tc.tile_pool(name="xin", bufs=6) as xin,
            tc.tile_pool(name="wk", bufs=1) as wk,
        ):
            # ---------------- P1: load + per-half extraction ----------------
            v16 = wk.tile([128, 16 * NIMG], f32)
            i16t = wk.tile([128, 16 * NIMG], u16)
            for m in range(NIMG):
                x = xin.tile([128, 625], f32, name=f"x{m}", tag="x")
                dq().dma_start(out=x[:], in_=d_log[m])
                cv = v16[:, 16 * m : 16 * m + 16]
                ci = i16t[:, 16 * m : 16 * m + 16]
                nc.vector.max(cv[:, 0:8], x[:, 0:HALF])
                nc.vector.max(cv[:, 8:16], x[:, HALF:625])
                nc.vector.max_index(ci[:, 0:8], cv[:, 0:8], x[:, 0:HALF])
                nc.vector.max_index(ci[:, 8:16], cv[:, 8:16], x[:, HALF:625])
            i16f = wk.tile([128, 16 * NIMG], f32)
            nc.vector.tensor_copy(i16f[:], i16t[:])

            # ---------------- P2 ----------------
            v2a = wk.tile([128, 512], f32)
            v2b = wk.tile([128, 512], f32)
            s2v = wk.tile([128, W2], f32)
            s2p = wk.tile([128, W2], u16)
            for m in range(NIMG):
                # [128,16] (p = g*32+pg, c) -> partitions 4m+g, free pg*16+c
                dq().dma_start(
                    out=v2a[4 * m : 4 * m + 4, :], in_=v16[:, 16 * m : 16 * m + 16]
                )
            for r in range(R2):
                cur = v2a if r % 2 == 0 else v2b
                nxt = v2b if r % 2 == 0 else v2a
                nc.vector.max(s2v[:, 8 * r : 8 * r + 8], cur[:])
                nc.vector.max_index(
                    s2p[:, 8 * r : 8 * r + 8], s2v[:, 8 * r : 8 * r + 8], cur[:]
                )
                nc.vector.match_replace(nxt[:], s2v[:, 8 * r : 8 * r + 8], cur[:], NEG)
            s2pf = wk.tile([128, W2], f32)
            nc.vector.tensor_copy(s2pf[:], s2p[:])

            # ---------------- P3 ----------------
            v3a = wk.tile([32, W3], f32)
            v3b = wk.tile([32, W3], f32)
            p448 = wk.tile([32, W3], f32)
            s3v = wk.tile([32, 8 * R3], f32)
            s3p = wk.tile([32, 8 * R3], u16)
            dq().dma_start(out=v3a[:], in_=s2v[:])
            dq().dma_start(out=p448[:], in_=s2pf[:])
            for r in range(R3):
                cur = v3a if r % 2 == 0 else v3b
                nxt = v3b if r % 2 == 0 else v3a
                nc.vector.max(s3v[:, 8 * r : 8 * r + 8], cur[:])
                nc.vector.max_index(
                    s3p[:, 8 * r : 8 * r + 8], s3v[:, 8 * r : 8 * r + 8], cur[:]
                )
                nc.vector.match_replace(nxt[:], s3v[:, 8 * r : 8 * r + 8], cur[:], NEG)

            def pi_i16(name, src_f32):
                """rank-order f32 [32, K_PAD] -> pi-permuted int16 for wrap DMAs."""
                pf = wk.tile([32, K_PAD], f32, name=f"{name}f")
                nc.vector.tensor_copy(pf[:], src_f32.rearrange("a (s x) -> a x s", x=16))
                pi = wk.tile([32, K_PAD], i16, name=f"{name}i")
                nc.vector.tensor_copy(pi[:], pf[:])
                return pi

            def floor_div(name, src, inv):
                t = wk.tile([32, K_PAD], f32, name=name)
                nc.vector.tensor_scalar(t[:], src[:], inv, DHALF, Alu.mult, Alu.add)
                nc.vector.tensor_scalar(t[:], t[:], MAGIC, None, Alu.add)
                nc.vector.tensor_scalar(t[:], t[:], MAGIC, None, Alu.subtract)
                return t

            # ---------------- EP gather 1: P3 pos -> P2 pos ----------------
            p3f = wk.tile([32, K_PAD], f32)
            nc.vector.tensor_copy(p3f[:], s3p[:])
            w1i = pi_i16("w1", p3f[:])
            e1 = wk.tile([32, K_PAD], f32)
            for cl in range(4):
                r0 = 8 * cl
                w1 = wk.tile([128, 19], i16, name=f"w1_{cl}")
                dq().dma_start(out=w1[:], in_=w1i[r0 : r0 + 8, :])
                t448 = wk.tile([128, W3], f32, name=f"t448_{cl}")
                nc.gpsimd.memset(t448[:], 0.0)
                dq().dma_start(out=t448[0:128:16, :], in_=p448[r0 : r0 + 8, :])
                o1 = wk.tile([128, K_PAD], f32, name=f"o1_{cl}")
                ap_gather(o1[:], t448[:], w1[:], num_elems=W3)
                dq().dma_start(out=e1[r0 : r0 + 8, :], in_=o1[0:128:16, :])

            # math-1: g = p3 // 112 ; s = g*512 + e1
            gg = floor_div("gg", p3f, 1.0 / W2)
            ss = wk.tile([32, K_PAD], f32)
            nc.vector.scalar_tensor_tensor(ss[:], gg[:], 512.0, e1[:], Alu.mult, Alu.add)
            w2i = pi_i16("w2", ss[:])

            # ---------------- EP gather 2: slot -> in-half index ----------------
            e2 = wk.tile([32, K_PAD], f32)
            for cl in range(4):
                r0 = 8 * cl
                w2 = wk.tile([128, 19], i16, name=f"w2_{cl}")
                dq().dma_start(out=w2[:], in_=w2i[r0 : r0 + 8, :])
                t1 = wk.tile([128, 2048], f32, name=f"t1_{cl}")
                nc.gpsimd.memset(t1[:], 0.0)
                for gi in range(8):
                    m = r0 + gi
                    dq().dma_start(
                        out=t1[16 * gi : 16 * gi + 1, :],
                        in_=i16f[:, 16 * m : 16 * m + 16],
                    )
                o2 = wk.tile([128, K_PAD], f32, name=f"o2_{cl}")
                ap_gather(o2[:], t1[:], w2[:], num_elems=2048)
                dq().dma_start(out=e2[r0 : r0 + 8, :], in_=o2[0:128:16, :])

            # math-2: flat = (s//16)*625 + (s%16>=8)*313 + j ; labels ; q
            pq = floor_div("pq", ss, 1.0 / 16.0)
            ccm = wk.tile([32, K_PAD], f32)
            nc.vector.scalar_tensor_tensor(
                ccm[:], pq[:], -16.0, ss[:], Alu.mult, Alu.add
            )
            hh = wk.tile([32, K_PAD], f32)
            nc.vector.tensor_scalar(hh[:], ccm[:], 8.0, None, Alu.is_ge)
            flat = wk.tile([32, K_PAD], f32)
            nc.vector.scalar_tensor_tensor(
                flat[:], pq[:], 625.0, e2[:], Alu.mult, Alu.add
            )
            nc.vector.scalar_tensor_tensor(
                flat[:], hh[:], float(HALF), flat[:], Alu.mult, Alu.add
            )
            qf = floor_div("qf", flat, 1.0 / C)
            lab = wk.tile([32, K_PAD], f32)
            nc.vector.scalar_tensor_tensor(
                lab[:], qf[:], -float(C), flat[:], Alu.mult, Alu.add
            )
            labi = wk.tile([32, K_PAD], i32)
            nc.vector.tensor_copy(labi[:], lab[:])
            dq().dma_start(out=d_lab[:], in_=labi[:])
            w3i = pi_i16("w3", qf[:])

            # scores
            sc = wk.tile([32, K_PAD], f32)
            nc.scalar.activation(sc[:], s3v[:], Act.Sigmoid)
            dq().dma_start(out=d_sco[:], in_=sc[:])

            # ---------------- EP: boxes ----------------
            cx = wk.tile([32, Q], f32)
            cy = wk.tile([32, Q], f32)
            wd = wk.tile([32, Q], f32)
            ht = wk.tile([32, Q], f32)
            nc.scalar.dma_start(out=cx[:], in_=d_box[:, :, 0])
            nc.scalar.dma_start(out=cy[:], in_=d_box[:, :, 1])
            nc.scalar.dma_start(out=wd[:], in_=d_box[:, :, 2])
            nc.scalar.dma_start(out=ht[:], in_=d_box[:, :, 3])
            sz = wk.tile([32, 2], f32)
            dq().dma_start(out=sz[:], in_=d_siz[:])
            pl = []
            for j, (cn, wh, sgn, sci) in enumerate(
                [(cx, wd, -0.5, 0), (cy, ht, -0.5, 1), (cx, wd, 0.5, 0), (cy, ht, 0.5, 1)]
            ):
                u = wk.tile([32, Q], f32, name=f"u{j}")
                nc.vector.scalar_tensor_tensor(
                    u[:], wh[:], sgn, cn[:], Alu.mult, Alu.add
                )
                v = wk.tile([32, Q], f32, name=f"pl{j}")
                nc.scalar.activation(v[:], u[:], Act.Copy, scale=sz[:, sci : sci + 1])
                pl.append(v)
            for cl in range(4):
                r0 = 8 * cl
                w3 = wk.tile([128, 19], i16, name=f"w3_{cl}")
                dq().dma_start(out=w3[:], in_=w3i[r0 : r0 + 8, :])
                pt = wk.tile([128, Q], f32, name=f"pt_{cl}")
                nc.gpsimd.memset(pt[:], 0.0)
                for j in range(4):
                    dq().dma_start(out=pt[j : 128 : 16, :], in_=pl[j][r0 : r0 + 8, :])
                o3 = wk.tile([128, K_PAD], f32, name=f"o3_{cl}")
                ap_gather(o3[:], pt[:], w3[:], num_elems=Q)
                for j in range(4):
                    dq().dma_start(
                        out=d_obx[r0 : r0 + 8, 0:K_OUT, j],
                        in_=o3[j : 128 : 16, 0:K_OUT],
                    )

    nc.compile()
    return nc


def _get_nc():
    if "nc" not in _CACHE:
        _CACHE["nc"] = _build()
    return _CACHE["nc"]


def kernel(pred_logits, pred_boxes, orig_target_sizes, num_top_queries):
    from concourse.bass_utils import run_bass_kernel_spmd

    assert int(num_top_queries) == K_OUT
    pred_logits = np.ascontiguousarray(np.asarray(pred_logits, dtype=np.float32))
    pred_boxes = np.ascontiguousarray(np.asarray(pred_boxes, dtype=np.float32))
    sizes = np.ascontiguousarray(np.asarray(orig_target_sizes, dtype=np.float32))

    nc = _get_nc()
    in_maps = []
    for c in range(NCORES):
        sl = slice(c * NIMG, (c + 1) * NIMG)
        in_maps.append(
            {
                "logits": pred_logits[sl].reshape(NIMG, 128, 625),
                "boxes": pred_boxes[sl],
                "sizes": sizes[sl],
            }
        )
    res = run_bass_kernel_spmd(nc, in_maps, list(range(NCORES))).results

    labels = np.concatenate([r["out_labels"][:, :K_OUT] for r in res], axis=0)
    scores = np.concatenate([r["out_scores"][:, :K_OUT] for r in res], axis=0)
    boxes = np.concatenate([r["out_boxes"] for r in res], axis=0)
    return labels.astype(np.int32), boxes, scores
